# revision 22
# baseline (speedup 1.0000x reference)
"""Distributed Trainium2 Bass kernel for the AllegroLayer GNN message-passing problem.

Sharding strategy (host side, inside kernel()): edges are partitioned across the
8 NeuronCores BY SOURCE-NODE RANGE (graph partitioning). Core k owns nodes
[3125k, 3125(k+1)) and every edge whose edge_src falls in that range. The
segment_sum then becomes fully core-local (no cross-core reduction at all), so
no collectives are needed; each core runs an independent single-core program.

Within a core, nodes are processed in windows of 128. All edges of a window are
host-sorted to be contiguous and padded to a whole number of 128-edge blocks.
The per-window segment sum is computed on the TensorEngine as
    agg[j, f] = sum_b  B_b.T @ msg_b        (PSUM accumulation across blocks)
where B_b[e, j] = (src_local[e] == j) is a 0/1 indicator built with a
tensor_scalar is_equal against an iota row. The gather-back (agg[src_e]) is the
transposed product  wY.T = agg.T B.T  computed as a second matmul per block.
No scatter/gather DMA is needed anywhere.

All PE compute runs in fp16 (exact-enough: ~5e-4 per rounding), accumulating in
fp32 PSUM. The e3nn CG product + 3-layer MLP are fused: the "scalars" channel
block never exists explicitly - the elementwise product P = wY * V feeds MLP
layer 1 through a host-refolded weight matrix.
"""

import numpy as np

# ---------------------------------------------------------------- constants
N_NODES = 25000
N_EDGES = 400000
FEAT = 128
N_CH = 32
N_CORES = 8
NODES_PER_CORE = N_NODES // N_CORES  # 3125
WIN = 128                            # nodes per window
N_WIN = (NODES_PER_CORE + WIN - 1) // WIN  # 25
INV_SQRT3 = 1.0 / np.sqrt(3.0)
NUM_NEIGHBORS = 16.0

# envelope poly coefficients for P=6 (poly_envelope(5, 2)): powers 6..8? ->
# computed exactly like the reference below in _envelope_coeffs().


def _envelope_coeffs(n0, n1):
    D = n0 + n1 + 1
    powers = np.arange(n0 + 1, D + 1)
    m = n1 + 1
    A = np.zeros((m, m)); b = np.zeros(m)
    for k in range(m):
        for ji, j in enumerate(powers):
            A[k, ji] = np.prod(np.arange(j, j - k, -1.0))
        b[k] = -1.0 if k == 0 else 0.0
    return powers, np.linalg.solve(A, b)

_POW, _COEF = _envelope_coeffs(5, 2)  # powers [6,7,8] with P=6 -> n0=5


# ---------------------------------------------------------------- host prep
def _prep(x, V, r, Y, edge_src):
    src = np.asarray(edge_src).astype(np.int64).ravel()
    order = np.argsort(src, kind="stable")
    ssrc = src[order]

    # window boundaries in node-id space, per (core, window)
    starts = []
    for k in range(N_CORES):
        base = NODES_PER_CORE * k
        for w in range(N_WIN):
            starts.append(base + min(WIN * w, NODES_PER_CORE))
    starts.append(N_NODES)
    bounds = np.searchsorted(ssrc, np.array(starts))
    counts = np.diff(bounds).reshape(N_CORES, N_WIN)

    NB = np.maximum(1, -(-counts.max(axis=0) // 128)).astype(int)  # blocks/window
    NBtot = int(NB.sum())
    SLOTS = NBtot * 128
    blk_win = np.repeat(np.arange(N_WIN), NB)          # window of global block
    win_blk0 = np.concatenate([[0], np.cumsum(NB)])    # first block of window

    # slot -> original edge index (-1 = padding)
    slot_edge = np.full((N_CORES, SLOTS), -1, dtype=np.int64)
    for k in range(N_CORES):
        for w in range(N_WIN):
            c = int(counts[k, w])
            b0 = int(bounds[k * N_WIN + w])
            s0 = int(win_blk0[w]) * 128
            slot_edge[k, s0:s0 + c] = order[b0:b0 + c]

    x = np.asarray(x); V = np.asarray(V); r = np.asarray(r); Y = np.asarray(Y)
    f16 = np.float16
    in_maps = []
    for k in range(N_CORES):
        se = slot_edge[k]
        valid = se >= 0
        sev = np.where(valid, se, 0)

        xk = np.where(valid[:, None], x[sev], 0).astype(f16)          # (SLOTS,128)
        vk = np.where(valid[:, None, None], V[sev], 0)                # (SLOTS,32,4)
        vk = np.ascontiguousarray(vk.transpose(0, 2, 1)).reshape(SLOTS, 128)
        vk = vk.astype(f16)                                           # m-major
        yk = np.where(valid[:, None], Y[sev], 0).astype(np.float32)   # (SLOTS,4)
        rk = np.where(valid[:, None], r[sev], 0).astype(np.float32)   # (SLOTS,3)

        # column layouts [128, cols] : col block Bg -> slot Bg*128 + p
        yc = yk.reshape(NBtot, 128, 4).transpose(1, 0, 2).reshape(128, 4 * NBtot)
        rc = rk.reshape(NBtot, 128, 3).transpose(1, 0, 2).reshape(128, 3 * NBtot)
        node0 = NODES_PER_CORE * k + blk_win * WIN                    # (NBtot,)
        srcloc = np.where(valid, src[sev] - np.repeat(node0, 128), 0)
        sc = srcloc.reshape(NBtot, 128).T.astype(np.float32)          # [128,NBtot]

        in_maps.append(dict(x=xk, v=vk, y=np.ascontiguousarray(yc),
                            r=np.ascontiguousarray(rc),
                            srcloc=np.ascontiguousarray(sc)))

    meta = dict(NB=[int(v) for v in NB], NBtot=NBtot, SLOTS=SLOTS,
                win_blk0=[int(v) for v in win_blk0], slot_edge=slot_edge)
    return in_maps, meta


def _weights(W1, W2a, W2b, W2c, Wlin):
    f16 = np.float16
    s192 = 1.0 / np.sqrt(192.0)
    w1 = (W1 / np.sqrt(128.0)).astype(f16)                   # [128,32]
    w2ax = (W2a[0:128] * s192).astype(f16)                   # [128,128]
    w2ap = np.zeros((128, 128), np.float32)                  # rows m*32+c
    for m in range(4):
        for c in range(N_CH):
            row = W2a[128 + 2 * c + (0 if m == 0 else 1)] * s192
            if m > 0:
                row = row * INV_SQRT3
            w2ap[m * 32 + c] = row
    w2ap = w2ap.astype(f16)
    w2b = (W2b / np.sqrt(128.0)).astype(f16)
    w2c = (W2c / np.sqrt(128.0)).astype(f16)
    # block-diagonal Wlin for the fused V_out matmuls:
    #   vop[e, (i,v)] = sum_{(i',c)} Vv1[(i',c),e]*BD1[(i',c),(i,v)]
    #                 + sum_{(i',c)} Vv2[(i',c),e]*BD2[(i',c),(i,v)]
    # Vv1[(i,c)] = a_s[c]*b_v[c,i]  -> coeff Wlin[2c]/8
    # Vv2[(i,c)] = a_v[c,i]*b_s[c]  -> coeff Wlin[2c+1]/8
    bd1 = np.zeros((96, 96), np.float32)
    bd2 = np.zeros((96, 96), np.float32)
    for i in range(3):
        for c in range(N_CH):
            bd1[i * 32 + c, i * 32:(i + 1) * 32] = Wlin[2 * c] / 8.0
            bd2[i * 32 + c, i * 32:(i + 1) * 32] = Wlin[2 * c + 1] / 8.0
    bd1 = bd1.astype(f16); bd2 = bd2.astype(f16)
    ident = np.ascontiguousarray(np.eye(128, dtype=f16))
    iota = np.ascontiguousarray(
        np.tile(np.arange(128, dtype=f16)[None, :], (128, 1)))
    return dict(w1=w1, w2ax=w2ax, w2ap=w2ap, w2b=w2b, w2c=w2c,
                wlbd1=bd1, wlbd2=bd2, ident=ident, iota=iota)


# ---------------------------------------------------------------- device build
def _build(meta):
    import concourse.bass as bass
    import concourse.mybir as mybir
    import concourse.tile as tile
    import concourse.bacc as bacc

    F32 = mybir.dt.float32
    F16 = mybir.dt.float16
    AF = mybir.ActivationFunctionType
    ALU = mybir.AluOpType

    NB = meta["NB"]; NBtot = meta["NBtot"]; SLOTS = meta["SLOTS"]
    win_blk0 = meta["win_blk0"]

    nc = bacc.Bacc("TRN2", target_bir_lowering=False, debug=False,
                   num_devices=N_CORES)

    x_d = nc.dram_tensor("x", [SLOTS, 128], F16, kind="ExternalInput")
    v_d = nc.dram_tensor("v", [SLOTS, 128], F16, kind="ExternalInput")
    y_d = nc.dram_tensor("y", [128, 4 * NBtot], F32, kind="ExternalInput")
    r_d = nc.dram_tensor("r", [128, 3 * NBtot], F32, kind="ExternalInput")
    sl_d = nc.dram_tensor("srcloc", [128, NBtot], F32, kind="ExternalInput")
    w1_d = nc.dram_tensor("w1", [128, 32], F16, kind="ExternalInput")
    w2ax_d = nc.dram_tensor("w2ax", [128, 128], F16, kind="ExternalInput")
    w2ap_d = nc.dram_tensor("w2ap", [128, 128], F16, kind="ExternalInput")
    w2b_d = nc.dram_tensor("w2b", [128, 128], F16, kind="ExternalInput")
    w2c_d = nc.dram_tensor("w2c", [128, 128], F16, kind="ExternalInput")
    wlbd1_d = nc.dram_tensor("wlbd1", [96, 96], F16, kind="ExternalInput")
    wlbd2_d = nc.dram_tensor("wlbd2", [96, 96], F16, kind="ExternalInput")
    id_d = nc.dram_tensor("ident", [128, 128], F16, kind="ExternalInput")
    iota_d = nc.dram_tensor("iota", [128, 128], F16, kind="ExternalInput")
    xo_d = nc.dram_tensor("xo", [SLOTS, 128], F32, kind="ExternalOutput")
    vo_d = nc.dram_tensor("vo", [SLOTS, 96], F32, kind="ExternalOutput")

    x_blk = x_d[:].rearrange("(b p) k -> p b k", p=128)   # [128, NBtot, 128]
    v_blk = v_d[:].rearrange("(b p) k -> p b k", p=128)
    xo_blk = xo_d[:].rearrange("(b p) k -> p b k", p=128)
    vo_blk = vo_d[:].rearrange("(b p) k -> p b k", p=128)  # [128, NBtot, 96]

    c5, c6, c7 = [float(v) for v in _COEF]  # coeffs for powers 6,7,8
    p0, p1, p2 = [int(v) for v in _POW]
    assert (p0, p1, p2) == (6, 7, 8)

    with tile.TileContext(nc) as tc:
        with tc.tile_pool(name="const", bufs=1) as cpool, \
             tc.tile_pool(name="ysl", bufs=1) as ypool, \
             tc.tile_pool(name="xt", bufs=12) as xtpool, \
             tc.tile_pool(name="bt", bufs=12) as btpool, \
             tc.tile_pool(name="io", bufs=4) as iopool, \
             tc.tile_pool(name="work", bufs=4) as wk, \
             tc.tile_pool(name="mlp", bufs=3) as mlppool, \
             tc.tile_pool(name="env", bufs=3) as envpool, \
             tc.tile_pool(name="ps_a", bufs=2, space="PSUM") as ps_a, \
             tc.tile_pool(name="ps_b", bufs=1, space="PSUM") as ps_b, \
             tc.tile_pool(name="ps_agg", bufs=1, space="PSUM") as ps_agg, \
             tc.tile_pool(name="ps_h", bufs=1, space="PSUM") as ps_h:

            # ---- constants
            w1 = cpool.tile([128, 32], F16, tag="w1")
            nc.sync.dma_start(w1[:], w1_d[:])
            w2ax = cpool.tile([128, 128], F16, tag="w2ax")
            nc.sync.dma_start(w2ax[:], w2ax_d[:])
            w2ap = cpool.tile([128, 128], F16, tag="w2ap")
            nc.sync.dma_start(w2ap[:], w2ap_d[:])
            w2b = cpool.tile([128, 128], F16, tag="w2b")
            nc.sync.dma_start(w2b[:], w2b_d[:])
            w2c = cpool.tile([128, 128], F16, tag="w2c")
            nc.sync.dma_start(w2c[:], w2c_d[:])
            wlbd1 = cpool.tile([96, 96], F16, tag="wlbd1")
            nc.sync.dma_start(wlbd1[:], wlbd1_d[:])
            wlbd2 = cpool.tile([96, 96], F16, tag="wlbd2")
            nc.sync.dma_start(wlbd2[:], wlbd2_d[:])
            ident = cpool.tile([128, 128], F16, tag="ident")
            nc.sync.dma_start(ident[:], id_d[:])
            iota = cpool.tile([128, 128], F16, tag="iota")
            nc.sync.dma_start(iota[:], iota_d[:])
            y_sb = ypool.tile([128, 4 * NBtot], F32, tag="ysb")
            nc.sync.dma_start(y_sb[:], y_d[:])
            sl_sb = ypool.tile([128, NBtot], F32, tag="slsb")
            nc.sync.dma_start(sl_sb[:], sl_d[:])

            for w in range(N_WIN):
                nb = NB[w]
                B0 = win_blk0[w]
                nst = -(-nb // 4)  # supertiles

                # ---------------- envelope for the window  [128, nb]
                rr = envpool.tile([128, 3 * nb], F32, tag="rr")
                nc.sync.dma_start(rr[:], r_d[:, 3 * B0:3 * (B0 + nb)])
                d2 = envpool.tile([128, nb], F32, tag="d2")
                sq = envpool.tile([128, 3 * nb], F32, tag="sq")
                nc.vector.tensor_tensor(sq[:], rr[:], rr[:], ALU.mult)
                nc.vector.tensor_reduce(
                    d2[:], sq[:].rearrange("p (n k) -> p n k", k=3),
                    mybir.AxisListType.X, ALU.add)
                d1 = envpool.tile([128, nb], F32, tag="d1")
                nc.scalar.activation(d1[:], d2[:], AF.Sqrt)
                d3 = envpool.tile([128, nb], F32, tag="d3")
                nc.vector.tensor_tensor(d3[:], d1[:], d2[:], ALU.mult)
                # q = c5 + c6*d + c7*d2   (coeffs of d^6,d^7,d^8 after factoring d^6)
                qa = envpool.tile([128, nb], F32, tag="qa")
                nc.vector.tensor_scalar(qa[:], d1[:], float(c6), float(c5),
                                        ALU.mult, ALU.add)
                qb = envpool.tile([128, nb], F32, tag="qb")
                nc.vector.tensor_scalar(qb[:], d2[:], float(c7), None, ALU.mult)
                nc.vector.tensor_tensor(qa[:], qa[:], qb[:], ALU.add)
                # env = 1 + d^6 * q ;  d^6 = d3*d3
                d6 = envpool.tile([128, nb], F32, tag="d6")
                nc.vector.tensor_tensor(d6[:], d3[:], d3[:], ALU.mult)
                env = envpool.tile([128, nb], F32, tag="env")
                nc.vector.tensor_tensor(env[:], d6[:], qa[:], ALU.mult)
                nc.vector.tensor_scalar(env[:], env[:], 1.0, None, ALU.add)
                mask = envpool.tile([128, nb], F32, tag="mask")
                nc.vector.tensor_scalar(mask[:], d1[:], 1.0, None, ALU.is_lt)
                nc.vector.tensor_tensor(env[:], env[:], mask[:], ALU.mult)

                # ---------------- phase A : aggregate into agg_psum
                agg_ps = ps_agg.tile([128, 128], F32, tag="agg")
                xts = []   # [128, 512] fp16 xT supertiles
                bts = []   # [128, 512] fp16 BT supertiles
                for s in range(nst):
                    nb4 = min(4, nb - 4 * s)
                    cw = 128 * nb4
                    Bg = B0 + 4 * s
                    xq = iopool.tile([128, 512], F16, tag="xq")
                    nc.sync.dma_start(
                        xq[:].rearrange("p (b k) -> p b k", k=128)[:, :nb4],
                        x_blk[:, Bg:Bg + nb4])
                    xtp = ps_a.tile([128, 512], F16, tag="tr")
                    for j in range(nb4):
                        nc.tensor.transpose(xtp[:, 128 * j:128 * (j + 1)],
                                            xq[:, 128 * j:128 * (j + 1)], ident[:])
                    xt = xtpool.tile([128, 512], F16, tag="xt")
                    nc.scalar.copy(xt[:, :cw], xtp[:, :cw])
                    xts.append(xt)

                    w4_ps = ps_b.tile([128, 128], F32, tag="w4")
                    for j in range(nb4):
                        nc.tensor.matmul(w4_ps[:, 32 * j:32 * (j + 1)],
                                         xt[:, 128 * j:128 * (j + 1)], w1[:],
                                         start=True, stop=True)
                    w4 = wk.tile([128, 128], F16, tag="w4sb")
                    nc.scalar.copy(w4[:, :32 * nb4], w4_ps[:, :32 * nb4])

                    btp = ps_b.tile([128, 512], F16, tag="btp")
                    for j in range(nb4):
                        b = 4 * s + j
                        msg = wk.tile([128, 128], F16, tag="msg")
                        for m in range(4):
                            nc.vector.tensor_scalar(
                                msg[:, 32 * m:32 * (m + 1)],
                                w4[:, 32 * j:32 * (j + 1)],
                                y_sb[:, 4 * (Bg + j) + m:4 * (Bg + j) + m + 1],
                                None, ALU.mult)
                        Bm = wk.tile([128, 128], F16, tag="Bm")
                        nc.vector.tensor_scalar(Bm[:], iota[:],
                                                sl_sb[:, Bg + j:Bg + j + 1],
                                                None, ALU.is_equal)
                        nc.tensor.matmul(agg_ps[:], Bm[:], msg[:],
                                         start=(b == 0), stop=(b == nb - 1),
                                         skip_group_check=True)
                        nc.tensor.transpose(btp[:, 128 * j:128 * (j + 1)],
                                            Bm[:], ident[:])
                    bt = btpool.tile([128, 512], F16, tag="bt")
                    nc.scalar.copy(bt[:, :cw], btp[:, :cw])
                    bts.append(bt)

                agg = wk.tile([128, 128], F16, tag="aggsb")
                nc.scalar.activation(agg[:], agg_ps[:], AF.Copy,
                                     scale=1.0 / np.sqrt(NUM_NEIGHBORS))

                # ---------------- phase D
                for s in range(nst):
                    nb4 = min(4, nb - 4 * s)
                    cw = 128 * nb4
                    Bg = B0 + 4 * s
                    # expansion: wY.T [feat, e]
                    wyp = ps_b.tile([128, 512], F32, tag="wyxo")
                    for j in range(nb4):
                        nc.tensor.matmul(wyp[:, 128 * j:128 * (j + 1)],
                                         agg[:], bts[s][:, 128 * j:128 * (j + 1)],
                                         start=True, stop=True)
                    wyt = wk.tile([128, 512], F16, tag="wyt")
                    nc.scalar.copy(wyt[:, :cw], wyp[:, :cw])

                    vq = iopool.tile([128, 512], F16, tag="vq")
                    nc.sync.dma_start(
                        vq[:].rearrange("p (b k) -> p b k", k=128)[:, :nb4],
                        v_blk[:, Bg:Bg + nb4])
                    vtp = ps_a.tile([128, 512], F16, tag="tr")
                    for j in range(nb4):
                        nc.tensor.transpose(vtp[:, 128 * j:128 * (j + 1)],
                                            vq[:, 128 * j:128 * (j + 1)], ident[:])
                    vt = wk.tile([128, 512], F16, tag="vt")
                    nc.scalar.copy(vt[:, :cw], vtp[:, :cw])

                    # P = wY * V  (feat-major, fp16)
                    pt = wk.tile([128, 512], F16, tag="pt")
                    nc.vector.tensor_tensor(pt[:, :cw], wyt[:, :cw], vt[:, :cw],
                                            ALU.mult)
                    # partition-aligned operand tiles for the CG cross terms
                    # (walrus requires equal start partitions on TT operands;
                    #  SBUF->SBUF DMA does the partition moves)
                    asr = wk.tile([96, 512], F16, tag="asr")   # a_s x3
                    bsr = wk.tile([96, 512], F16, tag="bsr")   # b_s x3
                    for q in range(3):
                        nc.sync.dma_start(asr[32 * q:32 * (q + 1), :cw],
                                          wyt[0:32, :cw])
                        nc.sync.dma_start(bsr[32 * q:32 * (q + 1), :cw],
                                          vt[0:32, :cw])
                    avs = wk.tile([96, 512], F16, tag="avs")   # a_v shifted
                    bvs = wk.tile([96, 512], F16, tag="bvs")   # b_v shifted
                    nc.sync.dma_start(avs[:, :cw], wyt[32:128, :cw])
                    nc.sync.dma_start(bvs[:, :cw], vt[32:128, :cw])
                    vv1 = wk.tile([96, 512], F16, tag="vv1")   # a_s*b_v
                    nc.vector.tensor_tensor(vv1[:, :cw], asr[:, :cw],
                                            bvs[:, :cw], ALU.mult)
                    vv2 = wk.tile([96, 512], F16, tag="vv2")   # a_v*b_s
                    nc.vector.tensor_tensor(vv2[:, :cw], avs[:, :cw],
                                            bsr[:, :cw], ALU.mult)

                    # MLP
                    h1p = ps_h.tile([128, 512], F32, tag="h")
                    nc.tensor.matmul(h1p[:, :cw], w2ax[:], xts[s][:, :cw],
                                     start=True, stop=False)
                    nc.tensor.matmul(h1p[:, :cw], w2ap[:], pt[:, :cw],
                                     start=False, stop=True)
                    h1 = mlppool.tile([128, 512], F16, tag="h1sb")
                    nc.scalar.activation(h1[:, :cw], h1p[:, :cw], AF.Silu)
                    h2p = ps_h.tile([128, 512], F32, tag="h")
                    nc.tensor.matmul(h2p[:, :cw], w2b[:], h1[:, :cw],
                                     start=True, stop=True)
                    h2 = mlppool.tile([128, 512], F16, tag="h2sb")
                    nc.scalar.activation(h2[:, :cw], h2p[:, :cw], AF.Silu)
                    h3p = ps_h.tile([128, 512], F32, tag="h")
                    nc.tensor.matmul(h3p[:, :cw], w2c[:], h2[:, :cw],
                                     start=True, stop=True)
                    h3 = mlppool.tile([128, 512], F16, tag="h3sb")
                    nc.vector.tensor_copy(h3[:, :cw], h3p[:, :cw])

                    # x_out = env * h3.T
                    xop = ps_b.tile([128, 512], F16, tag="wyxo")
                    for j in range(nb4):
                        nc.tensor.transpose(xop[:, 128 * j:128 * (j + 1)],
                                            h3[:, 128 * j:128 * (j + 1)], ident[:])
                    xo = iopool.tile([128, 512], F32, tag="xo")
                    for j in range(nb4):
                        nc.scalar.activation(xo[:, 128 * j:128 * (j + 1)],
                                             xop[:, 128 * j:128 * (j + 1)],
                                             AF.Copy,
                                             scale=env[:, 4 * s + j:4 * s + j + 1])
                    nc.sync.dma_start(
                        xo_blk[:, Bg:Bg + nb4],
                        xo[:].rearrange("p (b k) -> p b k", k=128)[:, :nb4])

                    # V_out: accumulate the two block-diagonal matmuls
                    vop = ps_b.tile([128, 384], F32, tag="vop")
                    for j in range(nb4):
                        o0 = 96 * j
                        nc.tensor.matmul(vop[:, o0:o0 + 96],
                                         vv1[:, 128 * j:128 * (j + 1)],
                                         wlbd1[:], start=True, stop=False)
                        nc.tensor.matmul(vop[:, o0:o0 + 96],
                                         vv2[:, 128 * j:128 * (j + 1)],
                                         wlbd2[:], start=False, stop=True)
                    vo_sb = iopool.tile([128, 384], F32, tag="vosb")
                    nc.scalar.copy(vo_sb[:, :96 * nb4], vop[:, :96 * nb4])
                    nc.sync.dma_start(
                        vo_blk[:, Bg:Bg + nb4],
                        vo_sb[:].rearrange("p (b k) -> p b k", k=96)[:, :nb4])

    nc.compile()
    return nc


# ---------------------------------------------------------------- entry point
_CACHE = {}


def kernel(x, V, r, Y, edge_src, W1, W2a, W2b, W2c, Wlin):
    from concourse.bass_utils import run_bass_kernel_spmd

    in_maps, meta = _prep(x, V, r, Y, edge_src)
    wts = _weights(np.asarray(W1, np.float32), np.asarray(W2a, np.float32),
                   np.asarray(W2b, np.float32), np.asarray(W2c, np.float32),
                   np.asarray(Wlin, np.float32))
    for im in in_maps:
        im.update(wts)

    key = (meta["NBtot"], tuple(meta["NB"]))
    if key not in _CACHE:
        _CACHE[key] = _build(meta)
    nc = _CACHE[key]

    res = run_bass_kernel_spmd(nc, in_maps, core_ids=list(range(N_CORES)))

    SLOTS = meta["SLOTS"]
    x_out = np.empty((N_EDGES, 128), np.float32)
    V_out = np.empty((N_EDGES, 32, 3), np.float32)
    for k in range(N_CORES):
        se = meta["slot_edge"][k]
        valid = se >= 0
        idx = se[valid]
        xo = res.results[k]["xo"]
        vo = res.results[k]["vo"].reshape(SLOTS, 3, 32)
        x_out[idx] = xo[valid]
        V_out[idx] = vo[valid].transpose(0, 2, 1)
    return x_out, V_out


if __name__ == "__main__":
    import reference
    inputs = reference.setup_inputs()
    inputs = {k: np.asarray(v) for k, v in inputs.items()}
    out = kernel(**inputs)
    print("kernel ran:", out[0].shape, out[1].shape)
